# revision 3
# baseline (speedup 1.0000x reference)
"""BitLinear for Trainium2 — token-data-parallel bf16 redesign.

Math identity (same as reference):
    out = (x_int * scale) @ W_q.T + bias = scale[t] * (x_int @ W_q.T) + bias
with x_int in [-127,127] (exact in bf16) and W_q ternary (exact in bf16).
The GEMM runs in bf16 with fp32 PSUM accumulation -> exact integer math.

Sharding: DATA-parallel over tokens. Each of the 8 cores handles 1024
tokens x the FULL 16384 output features. Rationale vs the old
column-parallel layout:
  - per-core quantization work drops 8x (1024 tokens instead of 8192)
  - tokens live on PARTITIONS during quantization, so per-token absmax is
    a contiguous free-dim DVE reduce and the reciprocal/scale stay
    per-partition scalars: NO gpsimd partition reduce, no DRAM bounce
  - the per-token scale is already in PSUM-partition layout for the
    evacuation (psum partitions == tokens)
The quantized activations are DMA-XBAR-transposed (bf16, 2-byte: legal)
to K-major for the matmul; ternary W streams from HBM in 512-wide
out-feature chunks (double buffered), bf16, with FWL active (plain
matmul mode beats fp8 DoubleRow for exact 8-bit math: the hi/lo split
doubles MACs, cancelling DoubleRow's 2x and adding LDWEIGHTS penalty).

Engine budget per core: PE 8192 matmuls x ~213ns ~= 1.75 ms (the exact-
math roofline); DVE ~0.3 ms; ACT ~0.03 ms; DMA ~230 MB ~= 0.6 ms. All
non-PE work overlaps under the matmul stream.
"""

import sys

import numpy as np

if "/opt/trn_rl_repo" not in sys.path:
    sys.path.insert(0, "/opt/trn_rl_repo")

# ---------------------------------------------------------------- constants
B, T, D_IN, D_OUT = 4, 2048, 4096, 16384
NCORES = 8
NTOK = B * T                  # 8192 tokens
TPC = NTOK // NCORES          # 1024 tokens per core
P = 128                       # partitions
NGRP = TPC // P               # 8 token groups per core
KT = D_IN // P                # 32 k-tiles
OFCH = 512                    # out-feature chunk width (one PSUM bank)
NOFC = D_OUT // OFCH          # 32 chunks
MAX_INT = 127
EPS = 1e-8
THRESH = 0.5
MAGIC = 12582912.0            # 1.5 * 2**23: fp32 add/sub rounds to nearest int


def build_nc(xpose_mode="3d2", w_dt="fp8"):
    import concourse.mybir as mybir
    from concourse import bacc
    from concourse.tile import TileContext

    dt = mybir.dt
    alu = mybir.AluOpType
    wdt = dt.float8e4 if w_dt == "fp8" else dt.bfloat16

    nc = bacc.Bacc("TRN2", target_bir_lowering=False)
    x = nc.dram_tensor("x", [TPC, D_IN], dt.float32, kind="ExternalInput")
    # host-packed: wt[kp, ofc, kt, o'] = W_q[ofc*OFCH+o', k_of(kp,kt)]
    wt = nc.dram_tensor("wt", [P, NOFC, KT, OFCH], wdt,
                        kind="ExternalInput")
    bias = nc.dram_tensor("bias", [1, D_OUT], dt.float32, kind="ExternalInput")
    out = nc.dram_tensor("out", [TPC, D_OUT], dt.float32,
                         kind="ExternalOutput")

    with TileContext(nc) as tc_:
        with (
            tc_.tile_pool(name="res", bufs=1) as rpool,
            tc_.tile_pool(name="xg", bufs=2) as xpool,
            tc_.tile_pool(name="xq", bufs=2) as qpool,
            tc_.tile_pool(name="sc", bufs=2) as spool,
            tc_.tile_pool(name="w", bufs=2) as wpool,
            tc_.tile_pool(name="bb", bufs=2) as bpool,
            tc_.tile_pool(name="os", bufs=3) as opool,
            tc_.tile_pool(name="ps", bufs=2, space="PSUM") as ppool,
        ):
            # resident: K-major quantized activations + per-token scales
            xqT = rpool.tile([P, KT, TPC], dt.bfloat16, tag="xqT")
            s_all = rpool.tile([P, NGRP], dt.float32, tag="s_all")

            # ---- quantization: per 128-token group (token-major) --------
            with nc.named_scope("quant"):
                for g in range(NGRP):
                    xg = xpool.tile([P, D_IN], dt.float32, tag="xg",
                                    name=f"xg_{g}")
                    nc.sync.dma_start(xg[:], x[g * P:(g + 1) * P, :])
                    amax = spool.tile([P, 1], dt.float32, tag="amax",
                                      name=f"amax_{g}")
                    nc.vector.tensor_reduce(
                        amax[:], xg[:], axis=mybir.AxisListType.X,
                        op=alu.max, apply_absolute_value=True,
                    )
                    d = spool.tile([P, 1], dt.float32, tag="d", name=f"d_{g}")
                    nc.vector.tensor_scalar(
                        d[:], amax[:], 1.0 / MAX_INT, EPS, alu.mult, alu.add
                    )
                    r = spool.tile([P, 1], dt.float32, tag="r", name=f"r_{g}")
                    nc.vector.reciprocal(r[:], d[:])
                    nc.vector.tensor_scalar(
                        s_all[:, g:g + 1], amax[:], 1.0 / MAX_INT, None,
                        alu.mult,
                    )
                    # in place on ACT: xg = xg*r + MAGIC  (fp32, RNE rounds)
                    nc.scalar.activation(
                        xg[:], xg[:], mybir.ActivationFunctionType.Copy,
                        bias=MAGIC, scale=r[:],
                    )
                    xq = qpool.tile([P, D_IN], dt.bfloat16, tag="xq",
                                    name=f"xq_{g}")
                    nc.vector.tensor_scalar(
                        xq[:], xg[:], MAGIC, None, alu.subtract
                    )
                    # transpose to K-major. XBAR transposes are only
                    # correct on the sync (SP) queue — never scalar.
                    if xpose_mode in ("3d2", "3d1s"):
                        h = KT // 2
                        eng2 = nc.sync if xpose_mode == "3d1s" else nc.scalar
                        nc.sync.dma_start(
                            xqT[:, 0:h, g * P:(g + 1) * P],
                            xq[:, 0:h * P],
                            transpose=True,
                        )
                        eng2.dma_start(
                            xqT[:, h:KT, g * P:(g + 1) * P],
                            xq[:, h * P:KT * P],
                            transpose=True,
                        )
                    else:
                        for kt in range(KT):
                            eng = (nc.sync if (xpose_mode == "perk1"
                                               or kt % 2 == 0) else nc.scalar)
                            eng.dma_start(
                                xqT[:, kt, g * P:(g + 1) * P],
                                xq[:, kt * P:(kt + 1) * P],
                                transpose=True,
                            )

            # ---- GEMM: stream W chunks, psum per (ofc, token-group) -----
            with nc.named_scope("gemm"):
                for ofc in range(NOFC):
                    wbuf = wpool.tile([P, KT, OFCH], wdt, tag="w",
                                      name=f"w_{ofc}")
                    for sp in range(4):
                        ks = slice(sp * (KT // 4), (sp + 1) * (KT // 4))
                        nc.scalar.dma_start(wbuf[:, ks, :], wt[:, ofc, ks, :])
                    bb = bpool.tile([P, OFCH], dt.float32, tag="bb",
                                    name=f"bb_{ofc}")
                    nc.scalar.dma_start(
                        bb[:],
                        bias[0:1, ofc * OFCH:(ofc + 1) * OFCH]
                        .to_broadcast((P, OFCH)),
                    )
                    for tt in range(NGRP):
                        ps = ppool.tile([P, OFCH], dt.float32, tag="ps",
                                        name=f"ps_{ofc}_{tt}")
                        for k in range(KT):
                            nc.tensor.matmul(
                                ps, xqT[:, k, tt * P:(tt + 1) * P],
                                wbuf[:, k, :],
                                start=(k == 0), stop=(k == KT - 1),
                            )
                        osb = opool.tile([P, OFCH], dt.float32, tag="osb",
                                         name=f"osb_{ofc}_{tt}")
                        nc.vector.scalar_tensor_tensor(
                            osb[:], ps, s_all[:, tt:tt + 1], bb[:],
                            alu.mult, alu.add,
                        )
                        nc.scalar.dma_start(
                            out[tt * P:(tt + 1) * P,
                                ofc * OFCH:(ofc + 1) * OFCH],
                            osb[:],
                        )

    nc.finalize()
    return nc


# ------------------------------------------------------------------ host side
def _ternarize_weight(weight):
    """Reproduce the reference's forward weight path exactly, then snap the
    +-1ulp STE noise back to exact ternary by casting."""
    try:
        import jax
        import jax.numpy as jnp

        with jax.default_device(jax.devices("cpu")[0]):
            w = jnp.asarray(weight)
            w_scale = jnp.mean(jnp.abs(w))
            w_scaled = w / (w_scale + EPS)
            w_q = jnp.sign(w_scaled) * (jnp.abs(w_scaled) > THRESH).astype(w.dtype)
            return np.asarray(w_q).astype(np.float32)
    except Exception:
        w = weight.astype(np.float32)
        w_scale = np.float32(np.mean(np.abs(w), dtype=np.float64))
        w_scaled = w / (w_scale + np.float32(EPS))
        return (np.sign(w_scaled) * (np.abs(w_scaled) > THRESH)).astype(np.float32)


def _k_index_map(conv):
    """kidx[kp, kt] = global k held at xqT[kp, kt] for the given DMA
    transpose row convention."""
    kp = np.arange(P)[:, None]
    kt = np.arange(KT)[None, :]
    if conv == "ktp":                 # k = kt*P + kp
        return kt * P + kp
    if conv == "pkt":                 # per 16-kt half: k = half*2048 + kp*16 + ktl
        half = kt // 16
        ktl = kt % 16
        return half * (16 * P) + kp * 16 + ktl
    raise ValueError(conv)


def _pack_weight(w_q, conv, w_dt):
    """(D_OUT, D_IN) fp32 ternary -> [P, NOFC, KT, OFCH] with
    wt[kp, ofc, kt, o'] = w_q[ofc*OFCH+o', kidx[kp,kt]]; each GEMM chunk
    wt[:, ofc] is then one contiguous run per partition. Ternary values
    are exact in both fp8e4 and bf16."""
    import ml_dtypes

    np_dt = ml_dtypes.float8_e4m3 if w_dt == "fp8" else ml_dtypes.bfloat16
    wT = np.ascontiguousarray(w_q.T)                    # (D_IN, D_OUT)
    kidx = _k_index_map(conv)
    arr = wT[kidx.reshape(-1), :].reshape(P, KT, NOFC, OFCH)
    arr = arr.transpose(0, 2, 1, 3)
    return np.ascontiguousarray(arr).astype(np_dt)


_NC_CACHE = {}
LAST_RESULTS = None


def kernel(x, weight, bias):
    import os

    from concourse.bass_utils import run_bass_kernel_spmd

    xpose_mode = os.environ.get("KERNEL_XPOSE_MODE", "perk1")
    conv = os.environ.get("KERNEL_XPOSE_CONV", "pkt")
    w_dt = os.environ.get("KERNEL_W_DT", "fp8")
    if xpose_mode in ("perk", "perk1"):
        conv = "ktp"
    if xpose_mode == "3d1s":
        conv = os.environ.get("KERNEL_XPOSE_CONV", "ktp")
    key = (xpose_mode, w_dt)
    if key not in _NC_CACHE:
        _NC_CACHE[key] = build_nc(xpose_mode, w_dt)
    nc = _NC_CACHE[key]

    x2d = np.ascontiguousarray(
        x.reshape(NTOK, D_IN).astype(np.float32, copy=False))
    w_q = _ternarize_weight(np.asarray(weight))
    wt = _pack_weight(w_q, conv, w_dt)
    bias_f = np.asarray(bias).astype(np.float32, copy=False).reshape(1, D_OUT)

    in_maps = []
    for c in range(NCORES):
        in_maps.append({
            "x": x2d[c * TPC:(c + 1) * TPC, :],
            "wt": wt,
            "bias": bias_f,
        })

    trace = bool(os.environ.get("KERNEL_TRACE"))
    res = run_bass_kernel_spmd(nc, in_maps, core_ids=list(range(NCORES)),
                               trace=trace)
    global LAST_RESULTS
    LAST_RESULTS = res
    outs = [np.asarray(res.results[c]["out"]) for c in range(NCORES)]
    full = np.concatenate(outs, axis=0)                 # (NTOK, D_OUT)
    return full.reshape(B, T, D_OUT).astype(np.float32, copy=False)
